# revision 8
# baseline (speedup 1.0000x reference)
"""Bahdanau attention on Trainium2 — 8-core data-parallel over batch.

Per core (8 batches): energy^T[o,s] = tanh(W_enc^T-matmul(enc^T) + dec_proj[o]),
scores = v . energy (PE matmul with M=1), softmax over s, context = w . enc.

Layouts:
  - Main matmul contracts over h, so both operands carry h on partitions:
    lhsT = W_enc^T tiles (transposed once at setup), rhs = enc^T tiles
    (bf16 xbar SBUF->SBUF DMA transposes of natural-layout tiles).
  - enc is loaded once (fp32 DRAM -> bf16 SBUF via casting SWDGE DMA); the
    natural bf16 tiles are kept for the context matmul (contracts over s).
  - energy^T keeps o on partitions so dec_proj enters as the ACT bias and
    tanh happens during the PSUM->SBUF move.
"""

import numpy as np

import concourse.bass as bass  # noqa: F401  (bass types referenced via tile/bacc)
import concourse.mybir as mybir
import concourse.tile as tile
from concourse import bacc
from concourse.bass_utils import run_bass_kernel_spmd

B, S, H = 64, 2048, 1024
N_CORES = 8
BL = B // N_CORES  # batches per core
P = 128
HC = H // P        # h (and o) chunks of 128
ST = 512           # s-tile (matmul free dim / PSUM bank)
NT = S // ST       # s-tiles per batch
SB = ST // P       # 128-blocks per s-tile
NSB = S // P       # 128-blocks per batch

f32 = mybir.dt.float32
bf16 = mybir.dt.bfloat16
AF = mybir.ActivationFunctionType
AX = mybir.AxisListType
OP = mybir.AluOpType


def _build_body(tc, enc, dec, w_enc, w_dec, v, ctx_out, w_out, repeat=1):
    nc = tc.nc
    with (
        tc.tile_pool(name="const", bufs=1) as const_pool,
        tc.tile_pool(name="stage", bufs=3) as stage_pool,
        tc.tile_pool(name="nat", bufs=24) as nat_pool,
        tc.tile_pool(name="enct", bufs=3) as enct_pool,
        tc.tile_pool(name="work", bufs=2) as work_pool,
        tc.tile_pool(name="dram", bufs=2, space="DRAM") as dram_pool,
        tc.tile_pool(name="psum", bufs=2, space="PSUM") as psum_pool,
    ):
        # ---- setup: W_enc^T / W_dec^T in bf16, dec^T, v, dec_proj ----
        # [h_r, o_block, h_chunk, o_j]; lhsT tile (hc, oi) = wt[:, oi, hc, :]
        w_encT = const_pool.tile([P, HC, HC, P], bf16)
        w_decT = const_pool.tile([P, HC, HC, P], bf16)
        for wt, wsrc in ((w_encT, w_enc), (w_decT, w_dec)):
            for oi in range(HC):
                wrow = stage_pool.tile([P, H], bf16, tag="wrow")
                # casting DMA: fp32 DRAM -> bf16 SBUF (SWDGE)
                nc.gpsimd.dma_start(out=wrow, in_=wsrc[oi * P:(oi + 1) * P, :])
                # batched xbar transpose, contiguous dst block
                nc.sync.dma_start(out=wt[:, oi], in_=wrow, transpose=True)

        decT = const_pool.tile([P, HC, BL], bf16)  # dec^T chunks
        for hc in range(HC):
            nc.gpsimd.dma_start(
                out=decT[:, hc, :],
                in_=dec[:, hc * P:(hc + 1) * P].rearrange("b h -> h b"),
            )
        v_sb = const_pool.tile([P, HC], bf16)
        nc.gpsimd.dma_start(out=v_sb, in_=v.rearrange("(c p) -> p c", p=P))

        # dec_proj[o, b] for all local batches, fp32 (used as tanh bias)
        dec_projT = const_pool.tile([P, HC, BL], f32)
        for oi in range(HC):
            pdp = psum_pool.tile([P, BL], f32, tag="pdp", bufs=1)
            for hc in range(HC):
                nc.tensor.matmul(
                    pdp,
                    lhsT=w_decT[:, oi, hc, :],
                    rhs=decT[:, hc, :],
                    start=(hc == 0),
                    stop=(hc == HC - 1),
                )
            nc.scalar.copy(dec_projT[:, oi, :], pdp)

        # ---- main loop over local batches ----
        for b in [b for _ in range(repeat) for b in range(BL)]:
            nat_tiles = []
            scores = work_pool.tile([1, S], f32, tag="scores")
            for t in range(NT):
                # [h_r, s_block, h_chunk, s_j]: each batched transpose below
                # writes encT[:, sblk] contiguously (HW xbar needs that).
                encT = enct_pool.tile([P, SB, HC, P], bf16, tag="encT")
                for sblk in range(SB):
                    nat = nat_pool.tile([P, H], bf16, tag="nat")
                    s0 = t * ST + sblk * P
                    # casting DMA fp32 -> bf16, natural [s, h] layout
                    nc.gpsimd.dma_start(out=nat, in_=enc[b, s0:s0 + P, :])
                    nat_tiles.append(nat)
                    # one xbar transpose per 128x1024 tile: out[p,c,j]=in[j,c*P+p]
                    nc.sync.dma_start(out=encT[:, sblk], in_=nat, transpose=True)
                psum_sc = psum_pool.tile([1, ST], f32, tag="psc")
                for oi in range(HC):
                    psum_e = psum_pool.tile([P, ST], f32, tag="pe")
                    for hc in range(HC):
                        nc.tensor.matmul(
                            psum_e,
                            lhsT=w_encT[:, oi, hc, :],
                            rhs=encT[:, :, hc, :],
                            start=(hc == 0),
                            stop=(hc == HC - 1),
                        )
                    energy = work_pool.tile([P, ST], bf16, tag="energy", bufs=4)
                    nc.scalar.activation(
                        energy, psum_e, AF.Tanh, bias=dec_projT[:, oi, b:b + 1]
                    )
                    nc.tensor.matmul(
                        psum_sc,
                        lhsT=v_sb[:, oi:oi + 1],
                        rhs=energy,
                        start=(oi == 0),
                        stop=(oi == HC - 1),
                    )
                nc.scalar.copy(scores[:, t * ST:(t + 1) * ST], psum_sc)

            # softmax over s for batch b
            negmax = work_pool.tile([1, 1], f32, tag="negmax")
            nc.vector.tensor_reduce(negmax, scores, axis=AX.X, op=OP.max, negate=True)
            probs = work_pool.tile([1, S], f32, tag="probs")
            ssum = work_pool.tile([1, 1], f32, tag="ssum")
            nc.scalar.activation(probs, scores, AF.Exp, bias=negmax, accum_out=ssum)
            rsum = work_pool.tile([1, 1], f32, tag="rsum")
            nc.vector.reciprocal(rsum, ssum)
            wrow = work_pool.tile([1, S], f32, tag="wrow_out")
            nc.vector.tensor_scalar_mul(wrow, probs, rsum)
            nc.gpsimd.dma_start(out=w_out[b:b + 1, :], in_=wrow)

            # transposed bf16 weights for the context matmul (via DRAM scratch)
            wscratch = dram_pool.tile([S], f32, tag="wscratch")
            nc.gpsimd.dma_start(
                out=wscratch.rearrange("(a s) -> a s", a=1), in_=wrow
            )
            wT = work_pool.tile([P, NSB], bf16, tag="wT")
            nc.gpsimd.dma_start(out=wT, in_=wscratch.rearrange("(c p) -> p c", p=P))

            ctx_row = work_pool.tile([1, H], f32, tag="ctx_row")
            for half in range(2):
                pcx = psum_pool.tile([1, ST], f32, tag="pcx")
                for si in range(NSB):
                    nc.tensor.matmul(
                        pcx,
                        lhsT=wT[:, si:si + 1],
                        rhs=nat_tiles[si][:, half * ST:(half + 1) * ST],
                        start=(si == 0),
                        stop=(si == NSB - 1),
                    )
                nc.scalar.copy(ctx_row[:, half * ST:(half + 1) * ST], pcx)
            nc.gpsimd.dma_start(out=ctx_out[b:b + 1, :], in_=ctx_row)


_NC = {}


def _get_nc(repeat=1):
    if repeat not in _NC:
        nc = bacc.Bacc("TRN2", target_bir_lowering=False, debug=False)
        enc = nc.dram_tensor("enc", (BL, S, H), f32, kind="ExternalInput").ap()
        dec = nc.dram_tensor("dec", (BL, H), f32, kind="ExternalInput").ap()
        w_enc = nc.dram_tensor("w_enc", (H, H), f32, kind="ExternalInput").ap()
        w_dec = nc.dram_tensor("w_dec", (H, H), f32, kind="ExternalInput").ap()
        v = nc.dram_tensor("v", (H,), f32, kind="ExternalInput").ap()
        ctx_out = nc.dram_tensor("ctx_out", (BL, H), f32, kind="ExternalOutput").ap()
        w_out = nc.dram_tensor("w_out", (BL, S), f32, kind="ExternalOutput").ap()
        with tile.TileContext(nc) as tc:
            _build_body(tc, enc, dec, w_enc, w_dec, v, ctx_out, w_out, repeat=repeat)
        nc.compile()
        _NC[repeat] = nc
    return _NC[repeat]


def run(inputs, **kwargs):
    """Run on 8 cores; returns (context, weights, BassKernelResults)."""
    enc = np.ascontiguousarray(np.asarray(inputs["enc_outputs"], dtype=np.float32))
    dec = np.ascontiguousarray(np.asarray(inputs["dec_hidden"], dtype=np.float32))
    w_enc = np.ascontiguousarray(np.asarray(inputs["W_enc"], dtype=np.float32))
    w_dec = np.ascontiguousarray(np.asarray(inputs["W_dec"], dtype=np.float32))
    v = np.ascontiguousarray(np.asarray(inputs["v"], dtype=np.float32))

    nc = _get_nc()
    in_maps = []
    for c in range(N_CORES):
        in_maps.append(
            {
                "enc": np.ascontiguousarray(enc[c * BL:(c + 1) * BL]),
                "dec": np.ascontiguousarray(dec[c * BL:(c + 1) * BL]),
                "w_enc": w_enc,
                "w_dec": w_dec,
                "v": v,
            }
        )
    res = run_bass_kernel_spmd(nc, in_maps, core_ids=list(range(N_CORES)), **kwargs)
    ctx = np.concatenate([r["ctx_out"] for r in res.results], axis=0)
    wts = np.concatenate([r["w_out"] for r in res.results], axis=0)
    return ctx, wts, res


def kernel(enc_outputs, dec_hidden, W_enc, W_dec, v):
    ctx, wts, _ = run(
        {
            "enc_outputs": enc_outputs,
            "dec_hidden": dec_hidden,
            "W_enc": W_enc,
            "W_dec": W_dec,
            "v": v,
        }
    )
    return ctx, wts


# revision 11
# speedup vs baseline: 1.8088x; 1.8088x over previous
"""Bahdanau attention on Trainium2 — 8-core data-parallel over batch.

Per core (8 batches): energy^T[o,s] = tanh(W_enc^T-matmul(enc^T) + dec_proj[o]),
scores = v . energy (PE matmul with M=1), softmax over s, context = w . enc.

Layouts:
  - Main matmul contracts over h, so both operands carry h on partitions:
    lhsT = W_enc^T tiles (transposed once at setup), rhs = enc^T tiles
    (bf16 xbar SBUF->SBUF DMA transposes of natural-layout tiles).
  - enc is loaded once (fp32 DRAM -> bf16 SBUF via casting SWDGE DMA); the
    natural bf16 tiles are kept for the context matmul (contracts over s).
  - energy^T keeps o on partitions so dec_proj enters as the ACT bias and
    tanh happens during the PSUM->SBUF move.
"""

import numpy as np

import concourse.bass as bass  # noqa: F401  (bass types referenced via tile/bacc)
import concourse.mybir as mybir
import concourse.tile as tile
from concourse import bacc
from concourse.bass_utils import run_bass_kernel_spmd

B, S, H = 64, 2048, 1024
N_CORES = 8
BL = B // N_CORES  # batches per core
P = 128
HC = H // P        # h (and o) chunks of 128
ST = 512           # s-tile (matmul free dim / PSUM bank)
NT = S // ST       # s-tiles per batch
SB = ST // P       # 128-blocks per s-tile
NSB = S // P       # 128-blocks per batch

f32 = mybir.dt.float32
bf16 = mybir.dt.bfloat16
AF = mybir.ActivationFunctionType
AX = mybir.AxisListType
OP = mybir.AluOpType


def _build_body(tc, enc, dec, w_enc, w_dec, v, ctx_out, w_out, repeat=1):
    nc = tc.nc
    with (
        tc.tile_pool(name="const", bufs=1) as const_pool,
        tc.tile_pool(name="stage", bufs=3) as stage_pool,
        tc.tile_pool(name="nat", bufs=2) as nat_pool,
        tc.tile_pool(name="enct", bufs=3) as enct_pool,
        tc.tile_pool(name="work", bufs=2) as work_pool,
        tc.tile_pool(name="dram", bufs=2, space="DRAM") as dram_pool,
        tc.tile_pool(name="psum", bufs=2, space="PSUM") as psum_pool,
    ):
        # ---- setup: W_enc^T / W_dec^T in bf16, dec^T, v, dec_proj ----
        # [h_r, o_block, h_chunk, o_j]; lhsT tile (hc, oi) = wt[:, oi, hc, :]
        w_encT = const_pool.tile([P, HC, HC, P], bf16)
        w_decT = const_pool.tile([P, HC, HC, P], bf16)
        for wt, wsrc in ((w_encT, w_enc), (w_decT, w_dec)):
            for oi in range(HC):
                wrow = stage_pool.tile([P, H], bf16, tag="wrow")
                # casting DMA: fp32 DRAM -> bf16 SBUF (SWDGE)
                nc.gpsimd.dma_start(out=wrow, in_=wsrc[oi * P:(oi + 1) * P, :])
                # batched xbar transpose, contiguous dst block
                nc.sync.dma_start(out=wt[:, oi], in_=wrow, transpose=True)

        decT = const_pool.tile([P, HC, BL], bf16)  # dec^T chunks
        for hc in range(HC):
            nc.gpsimd.dma_start(
                out=decT[:, hc, :],
                in_=dec[:, hc * P:(hc + 1) * P].rearrange("b h -> h b"),
            )
        v_sb = const_pool.tile([P, HC], bf16)
        nc.gpsimd.dma_start(out=v_sb, in_=v.rearrange("(c p) -> p c", p=P))

        # dec_proj[o, b] for all local batches, fp32 (used as tanh bias)
        dec_projT = const_pool.tile([P, HC, BL], f32)
        for oi in range(HC):
            pdp = psum_pool.tile([P, BL], f32, tag="pdp", bufs=1)
            for hc in range(HC):
                nc.tensor.matmul(
                    pdp,
                    lhsT=w_decT[:, oi, hc, :],
                    rhs=decT[:, hc, :],
                    start=(hc == 0),
                    stop=(hc == HC - 1),
                )
            nc.scalar.copy(dec_projT[:, oi, :], pdp)

        # ---- main loop over local batches ----
        for b in [b for _ in range(repeat) for b in range(BL)]:
            # whole batch in natural [s, h] layout, one DMA: nat[p, si, h]
            # = enc[b, si*128+p, h]; context matmul rhs slices come from here.
            nat = nat_pool.tile([P, NSB, H], bf16, tag="nat")
            nc.gpsimd.dma_start(
                out=nat, in_=enc[b].rearrange("(si p) h -> p si h", p=P)
            )
            scores = work_pool.tile([1, S], f32, tag="scores")
            for t in range(NT):
                # enc^T tile via one DRAM->SBUF xbar transpose:
                # encT[p, hc, j] = enc[b, t*ST+j, hc*128+p]
                encT = enct_pool.tile([P, HC, ST], bf16, tag="encT")
                nc.sync.dma_start(
                    out=encT, in_=enc[b, t * ST:(t + 1) * ST, :], transpose=True
                )
                psum_sc = psum_pool.tile([1, ST], f32, tag="psc")
                for oi in range(HC):
                    psum_e = psum_pool.tile([P, ST], f32, tag="pe")
                    for hc in range(HC):
                        nc.tensor.matmul(
                            psum_e,
                            lhsT=w_encT[:, oi, hc, :],
                            rhs=encT[:, hc, :],
                            start=(hc == 0),
                            stop=(hc == HC - 1),
                        )
                    energy = work_pool.tile([P, ST], bf16, tag="energy", bufs=4)
                    nc.scalar.activation(
                        energy, psum_e, AF.Tanh, bias=dec_projT[:, oi, b:b + 1]
                    )
                    nc.tensor.matmul(
                        psum_sc,
                        lhsT=v_sb[:, oi:oi + 1],
                        rhs=energy,
                        start=(oi == 0),
                        stop=(oi == HC - 1),
                    )
                nc.scalar.copy(scores[:, t * ST:(t + 1) * ST], psum_sc)

            # softmax over s for batch b
            negmax = work_pool.tile([1, 1], f32, tag="negmax")
            nc.vector.tensor_reduce(negmax, scores, axis=AX.X, op=OP.max, negate=True)
            probs = work_pool.tile([1, S], f32, tag="probs")
            ssum = work_pool.tile([1, 1], f32, tag="ssum")
            nc.scalar.activation(probs, scores, AF.Exp, bias=negmax, accum_out=ssum)
            rsum = work_pool.tile([1, 1], f32, tag="rsum")
            nc.vector.reciprocal(rsum, ssum)
            wrow = work_pool.tile([1, S], f32, tag="wrow_out")
            nc.vector.tensor_scalar_mul(wrow, probs, rsum)
            nc.gpsimd.dma_start(out=w_out[b:b + 1, :], in_=wrow)

            # transposed bf16 weights for the context matmul (via DRAM scratch)
            wscratch = dram_pool.tile([S], f32, tag="wscratch")
            nc.gpsimd.dma_start(
                out=wscratch.rearrange("(a s) -> a s", a=1), in_=wrow
            )
            wT = work_pool.tile([P, NSB], bf16, tag="wT")
            nc.gpsimd.dma_start(out=wT, in_=wscratch.rearrange("(c p) -> p c", p=P))

            ctx_row = work_pool.tile([1, H], f32, tag="ctx_row")
            for half in range(2):
                pcx = psum_pool.tile([1, ST], f32, tag="pcx")
                for si in range(NSB):
                    nc.tensor.matmul(
                        pcx,
                        lhsT=wT[:, si:si + 1],
                        rhs=nat[:, si, half * ST:(half + 1) * ST],
                        start=(si == 0),
                        stop=(si == NSB - 1),
                    )
                nc.scalar.copy(ctx_row[:, half * ST:(half + 1) * ST], pcx)
            nc.gpsimd.dma_start(out=ctx_out[b:b + 1, :], in_=ctx_row)


_NC = {}


def _get_nc(repeat=1):
    if repeat not in _NC:
        nc = bacc.Bacc("TRN2", target_bir_lowering=False, debug=False)
        enc = nc.dram_tensor("enc", (BL, S, H), bf16, kind="ExternalInput").ap()
        dec = nc.dram_tensor("dec", (BL, H), f32, kind="ExternalInput").ap()
        w_enc = nc.dram_tensor("w_enc", (H, H), f32, kind="ExternalInput").ap()
        w_dec = nc.dram_tensor("w_dec", (H, H), f32, kind="ExternalInput").ap()
        v = nc.dram_tensor("v", (H,), f32, kind="ExternalInput").ap()
        ctx_out = nc.dram_tensor("ctx_out", (BL, H), f32, kind="ExternalOutput").ap()
        w_out = nc.dram_tensor("w_out", (BL, S), f32, kind="ExternalOutput").ap()
        with tile.TileContext(nc) as tc:
            _build_body(tc, enc, dec, w_enc, w_dec, v, ctx_out, w_out, repeat=repeat)
        nc.compile()
        _NC[repeat] = nc
    return _NC[repeat]


def run(inputs, **kwargs):
    """Run on 8 cores; returns (context, weights, BassKernelResults)."""
    import ml_dtypes

    enc = np.ascontiguousarray(
        np.asarray(inputs["enc_outputs"]).astype(ml_dtypes.bfloat16)
    )
    dec = np.ascontiguousarray(np.asarray(inputs["dec_hidden"], dtype=np.float32))
    w_enc = np.ascontiguousarray(np.asarray(inputs["W_enc"], dtype=np.float32))
    w_dec = np.ascontiguousarray(np.asarray(inputs["W_dec"], dtype=np.float32))
    v = np.ascontiguousarray(np.asarray(inputs["v"], dtype=np.float32))

    nc = _get_nc()
    in_maps = []
    for c in range(N_CORES):
        in_maps.append(
            {
                "enc": np.ascontiguousarray(enc[c * BL:(c + 1) * BL]),
                "dec": np.ascontiguousarray(dec[c * BL:(c + 1) * BL]),
                "w_enc": w_enc,
                "w_dec": w_dec,
                "v": v,
            }
        )
    res = run_bass_kernel_spmd(nc, in_maps, core_ids=list(range(N_CORES)), **kwargs)
    ctx = np.concatenate([r["ctx_out"] for r in res.results], axis=0)
    wts = np.concatenate([r["w_out"] for r in res.results], axis=0)
    return ctx, wts, res


def kernel(enc_outputs, dec_hidden, W_enc, W_dec, v):
    ctx, wts, _ = run(
        {
            "enc_outputs": enc_outputs,
            "dec_hidden": dec_hidden,
            "W_enc": W_enc,
            "W_dec": W_dec,
            "v": v,
        }
    )
    return ctx, wts


# revision 14
# speedup vs baseline: 57080.2024x; 31556.4244x over previous
"""Bahdanau attention on Trainium2 — 8-core data-parallel over batch.

Per core (8 batches): energy^T[o,s] = tanh(W_enc^T-matmul(enc^T) + dec_proj[o]),
scores = v . energy (PE matmul with M=1), softmax over s, context = w . enc.

Layouts:
  - Main matmul contracts over h, so both operands carry h on partitions:
    lhsT = W_enc^T tiles (transposed once at setup), rhs = enc^T tiles
    (one bf16 xbar DRAM->SBUF DMA transpose per s-tile).
  - enc arrives pre-cast to bf16 (host side); it is read twice from HBM:
    transposed (xbar) for the energy matmul, natural for the context matmul.
  - energy^T keeps o on partitions so dec_proj enters as the ACT bias and
    tanh happens during the PSUM->SBUF move.
"""

import numpy as np

import concourse.bass as bass  # noqa: F401  (bass types referenced via tile/bacc)
import concourse.mybir as mybir
import concourse.tile as tile
from concourse import bacc
from concourse.bass_utils import run_bass_kernel_spmd

B, S, H = 64, 2048, 1024
N_CORES = 8
BL = B // N_CORES  # batches per core
P = 128
HC = H // P        # h (and o) chunks of 128
ST = 512           # s-tile (matmul free dim / PSUM bank)
NT = S // ST       # s-tiles per batch
SB = ST // P       # 128-blocks per s-tile
NSB = S // P       # 128-blocks per batch

f32 = mybir.dt.float32
bf16 = mybir.dt.bfloat16
AF = mybir.ActivationFunctionType
AX = mybir.AxisListType
OP = mybir.AluOpType


def _build_body(tc, enc, dec, w_enc, w_dec, v, ctx_out, w_out, repeat=1):
    nc = tc.nc
    with (
        tc.tile_pool(name="const", bufs=1) as const_pool,
        tc.tile_pool(name="stage", bufs=3) as stage_pool,
        tc.tile_pool(name="nat", bufs=2) as nat_pool,
        tc.tile_pool(name="enct", bufs=3) as enct_pool,
        tc.tile_pool(name="work", bufs=2) as work_pool,
        tc.tile_pool(name="dram", bufs=2, space="DRAM") as dram_pool,
        tc.tile_pool(name="psum", bufs=2, space="PSUM") as psum_pool,
    ):
        # ---- setup: W_enc^T / W_dec^T in bf16, dec^T, v, dec_proj ----
        # [h_r, o_block, h_chunk, o_j]; lhsT tile (hc, oi) = wt[:, oi, hc, :]
        w_encT = const_pool.tile([P, HC, HC, P], bf16)
        w_decT = const_pool.tile([P, HC, HC, P], bf16)
        for wt, wsrc in ((w_encT, w_enc), (w_decT, w_dec)):
            for oi in range(HC):
                wrow = stage_pool.tile([P, H], bf16, tag="wrow")
                # casting DMA: fp32 DRAM -> bf16 SBUF (SWDGE)
                nc.gpsimd.dma_start(out=wrow, in_=wsrc[oi * P:(oi + 1) * P, :])
                # batched xbar transpose, contiguous dst block
                nc.sync.dma_start(out=wt[:, oi], in_=wrow, transpose=True)

        decT = const_pool.tile([P, HC, BL], bf16)  # dec^T chunks
        for hc in range(HC):
            nc.gpsimd.dma_start(
                out=decT[:, hc, :],
                in_=dec[:, hc * P:(hc + 1) * P].rearrange("b h -> h b"),
            )
        v_sb = const_pool.tile([P, HC], bf16)
        nc.gpsimd.dma_start(out=v_sb, in_=v.rearrange("(c p) -> p c", p=P))

        # dec_proj[o, b] for all local batches, fp32 (used as tanh bias)
        dec_projT = const_pool.tile([P, HC, BL], f32)
        for oi in range(HC):
            pdp = psum_pool.tile([P, BL], f32, tag="pdp", bufs=1)
            for hc in range(HC):
                nc.tensor.matmul(
                    pdp,
                    lhsT=w_decT[:, oi, hc, :],
                    rhs=decT[:, hc, :],
                    start=(hc == 0),
                    stop=(hc == HC - 1),
                )
            nc.vector.tensor_copy(dec_projT[:, oi, :], pdp)

        # ---- main loop over local batches ----
        for b in [b for _ in range(repeat) for b in range(BL)]:
            # whole batch in natural [s, h] layout, one DMA: nat[p, si, h]
            # = enc[b, si*128+p, h]; context matmul rhs slices come from here.
            nat = nat_pool.tile([P, NSB, H], bf16, tag="nat")
            nc.gpsimd.dma_start(
                out=nat, in_=enc[b].rearrange("(si p) h -> p si h", p=P)
            )
            scores = work_pool.tile([1, S], f32, tag="scores")
            for tp in range(NT // 2):
                # s-tile pair (2*tp, 2*tp+1); each enc^T tile from one
                # DRAM->SBUF xbar transpose: encT[p, hc, j] = enc[b, s0+j, hc*128+p]
                encTs = []
                for half in range(2):
                    t = 2 * tp + half
                    encT = enct_pool.tile([P, HC, ST], bf16, tag="encT")
                    nc.sync.dma_start(
                        out=encT, in_=enc[b, t * ST:(t + 1) * ST, :], transpose=True
                    )
                    encTs.append(encT)
                psc = [
                    psum_pool.tile([1, ST], f32, tag="psc", bufs=3, name=f"psc{h}")
                    for h in range(2)
                ]
                for oi in range(HC):
                    # two PSUM banks per oi: cols [0:512) <- s-tile a, [512:1024) <- b
                    psum_e = psum_pool.tile([P, 2 * ST], f32, tag="pe")
                    for half in range(2):
                        for hc in range(HC):
                            nc.tensor.matmul(
                                psum_e[:, half * ST:(half + 1) * ST],
                                lhsT=w_encT[:, oi, hc, :],
                                rhs=encTs[half][:, hc, :],
                                start=(hc == 0),
                                stop=(hc == HC - 1),
                            )
                    # one tanh covers both banks; bias is the same oi chunk
                    energy = work_pool.tile([P, 2 * ST], bf16, tag="energy", bufs=4)
                    nc.scalar.activation(
                        energy, psum_e, AF.Tanh, bias=dec_projT[:, oi, b:b + 1]
                    )
                    for half in range(2):
                        nc.tensor.matmul(
                            psc[half],
                            lhsT=v_sb[:, oi:oi + 1],
                            rhs=energy[:, half * ST:(half + 1) * ST],
                            start=(oi == 0),
                            stop=(oi == HC - 1),
                        )
                for half in range(2):
                    t = 2 * tp + half
                    nc.vector.tensor_copy(
                        scores[:, t * ST:(t + 1) * ST], psc[half]
                    )

            # softmax over s for batch b
            negmax = work_pool.tile([1, 1], f32, tag="negmax")
            nc.vector.tensor_reduce(negmax, scores, axis=AX.X, op=OP.max, negate=True)
            probs = work_pool.tile([1, S], f32, tag="probs")
            ssum = work_pool.tile([1, 1], f32, tag="ssum")
            nc.scalar.activation(probs, scores, AF.Exp, bias=negmax, accum_out=ssum)
            rsum = work_pool.tile([1, 1], f32, tag="rsum")
            nc.vector.reciprocal(rsum, ssum)
            wrow = work_pool.tile([1, S], f32, tag="wrow_out")
            nc.vector.tensor_scalar_mul(wrow, probs, rsum)
            nc.gpsimd.dma_start(out=w_out[b:b + 1, :], in_=wrow)

            # transposed bf16 weights for the context matmul (via DRAM scratch)
            wscratch = dram_pool.tile([S], f32, tag="wscratch")
            nc.gpsimd.dma_start(
                out=wscratch.rearrange("(a s) -> a s", a=1), in_=wrow
            )
            wT = work_pool.tile([P, NSB], bf16, tag="wT")
            nc.gpsimd.dma_start(out=wT, in_=wscratch.rearrange("(c p) -> p c", p=P))

            ctx_row = work_pool.tile([1, H], f32, tag="ctx_row")
            for half in range(2):
                pcx = psum_pool.tile([1, ST], f32, tag="psc", bufs=3, name="pcx")
                for si in range(NSB):
                    nc.tensor.matmul(
                        pcx,
                        lhsT=wT[:, si:si + 1],
                        rhs=nat[:, si, half * ST:(half + 1) * ST],
                        start=(si == 0),
                        stop=(si == NSB - 1),
                    )
                nc.vector.tensor_copy(ctx_row[:, half * ST:(half + 1) * ST], pcx)
            nc.gpsimd.dma_start(out=ctx_out[b:b + 1, :], in_=ctx_row)


_NC = {}


def _get_nc(repeat=1):
    if repeat not in _NC:
        nc = bacc.Bacc("TRN2", target_bir_lowering=False, debug=False)
        enc = nc.dram_tensor("enc", (BL, S, H), bf16, kind="ExternalInput").ap()
        dec = nc.dram_tensor("dec", (BL, H), f32, kind="ExternalInput").ap()
        w_enc = nc.dram_tensor("w_enc", (H, H), f32, kind="ExternalInput").ap()
        w_dec = nc.dram_tensor("w_dec", (H, H), f32, kind="ExternalInput").ap()
        v = nc.dram_tensor("v", (H,), f32, kind="ExternalInput").ap()
        ctx_out = nc.dram_tensor("ctx_out", (BL, H), f32, kind="ExternalOutput").ap()
        w_out = nc.dram_tensor("w_out", (BL, S), f32, kind="ExternalOutput").ap()
        with tile.TileContext(nc) as tc:
            _build_body(tc, enc, dec, w_enc, w_dec, v, ctx_out, w_out, repeat=repeat)
        nc.compile()
        _NC[repeat] = nc
    return _NC[repeat]


def run(inputs, **kwargs):
    """Run on 8 cores; returns (context, weights, BassKernelResults)."""
    import ml_dtypes

    enc = np.ascontiguousarray(
        np.asarray(inputs["enc_outputs"]).astype(ml_dtypes.bfloat16)
    )
    dec = np.ascontiguousarray(np.asarray(inputs["dec_hidden"], dtype=np.float32))
    w_enc = np.ascontiguousarray(np.asarray(inputs["W_enc"], dtype=np.float32))
    w_dec = np.ascontiguousarray(np.asarray(inputs["W_dec"], dtype=np.float32))
    v = np.ascontiguousarray(np.asarray(inputs["v"], dtype=np.float32))

    nc = _get_nc()
    in_maps = []
    for c in range(N_CORES):
        in_maps.append(
            {
                "enc": np.ascontiguousarray(enc[c * BL:(c + 1) * BL]),
                "dec": np.ascontiguousarray(dec[c * BL:(c + 1) * BL]),
                "w_enc": w_enc,
                "w_dec": w_dec,
                "v": v,
            }
        )
    res = run_bass_kernel_spmd(nc, in_maps, core_ids=list(range(N_CORES)), **kwargs)
    ctx = np.concatenate([r["ctx_out"] for r in res.results], axis=0)
    wts = np.concatenate([r["w_out"] for r in res.results], axis=0)
    return ctx, wts, res


def kernel(enc_outputs, dec_hidden, W_enc, W_dec, v):
    ctx, wts, _ = run(
        {
            "enc_outputs": enc_outputs,
            "dec_hidden": dec_hidden,
            "W_enc": W_enc,
            "W_dec": W_dec,
            "v": v,
        }
    )
    return ctx, wts
